# revision 14
# baseline (speedup 1.0000x reference)
"""Trainium2 Bass kernel for nn_CrossAttentionLayer (sparse cross attention).

Sharding: 8 cores = 4 batches x 2 head-groups. Core c handles batch c//2 and
heads [4*(c%2), 4*(c%2)+4). Host compacts the key side through kv_mask (the
~50% masked-off keys are dropped and the survivors padded to a multiple of
128), so the device only attends over KP keys.

Device algorithm (per core):
  xlnT     = transpose(layernorm(x))        LN stats on DVE, apply on GPSIMD
                                            (q-side apply emits fp8 directly)
  qT8      = (Wq8.T @ xlnq8T) via fp8 DoubleRow matmuls, J-permuted columns
             so partition 32h+p holds head h, dim 32j+p at free slot j
  kT8      = Wk.T @ xlnkvT (bf16 matmuls, J-permuted), copied to fp8
  v        = xlnkvT.T @ Wv [k, 4h, 65]  (col 64 = 1.0 -> denominator)
  scores   = kT8.T @ qT8 per (head, k-tile, q-block) fp8 DoubleRow matmuls
             + 224*mask accumulated into the same PSUM region (fp8 DR matmul
             against a broadcast mask tile)
  pT       = exp(scores*SCALE - 28) on ACT -> masked-out entries ~e^-28
  acc      = pT.T-chunks @ [v|1]   [q 128, 4h, 65] accumulated over k-tiles
  a        = acc[:, :, 0:64] * (1/acc[:, :, 64])  per-partition normalize
  aT       = transpose(a) via PE; out = aT.T @ Wo  [q, E] partial, f32 to HBM
Host sums the two per-batch partials and adds bo.
"""

import os

import numpy as np
import ml_dtypes

import bass_rust
import concourse.bass as bass
import concourse.mybir as mybir
import concourse.tile as tile
from concourse import bass_utils
from concourse.vector_clock import ScopedClock


class _TileContext(tile.TileContext):
    """TileContext whose kernel-tail drain is split into single-wait drains.

    The walrus build in this environment rejects >1 sync-wait on a Drain
    (CTRL_NO struct): "Too many sync wait commands". The stock
    _drain_and_barrier attaches one wait per outstanding semaphore to a
    single Drain; emit one Drain per wait instead.
    """

    def _drain_and_barrier(self, tick_clock, wait_clock):
        drain_inst = self.nc.sync.drain()
        wait_clock.add_sem_waits(
            drain_inst.ins, ScopedClock({None: tick_clock.global_clock})
        )
        si = drain_inst.ins.sync_info
        if si is not None and si.on_wait and len(si.on_wait) > 1:
            waits = list(si.on_wait)
            drain_inst.ins.sync_info = bass_rust.SyncInfo(
                on_wait=[waits[0]], on_update=si.on_update or [])
            for w in waits[1:]:
                extra = self.nc.sync.drain()
                extra.ins.sync_info = bass_rust.SyncInfo(
                    on_wait=[w], on_update=[])

        self.nc.all_engine_barrier()
        assert self.sems is not None
        popped = self.nc._tile_sem_poison_stack.pop()
        assert popped is self._sem_poison
        self.nc.clear_and_free_semaphores(list(self.sems.allocated().values()))
        self.nc.all_engine_barrier()


def _split_sync_waits(nc):
    """Cap every instruction at one sync wait (walrus build limitation)."""
    for f in nc.m.functions:
        for bb in f.blocks:
            insns = bb.instructions
            out = []
            changed = False
            for ins in insns:
                si = ins.sync_info
                if si is not None and si.on_wait and len(si.on_wait) > 1:
                    waits = list(si.on_wait)
                    for w in waits[:-1]:
                        nop = mybir.InstNoOp(
                            name=nc.get_next_instruction_name(),
                            engine=ins.engine,
                            ins=[], outs=[],
                            sync_info=bass_rust.SyncInfo(
                                on_wait=[w], on_update=[]),
                        )
                        out.append(nop)
                    ins.sync_info = bass_rust.SyncInfo(
                        on_wait=[waits[-1]], on_update=si.on_update or [])
                    changed = True
                out.append(ins)
            if changed:
                bb.instructions = out


BF16 = ml_dtypes.bfloat16
FP8 = ml_dtypes.float8_e4m3

E = 512
H = 8
D = 64
TQ = 2048          # query tokens
P = 128
NQT = TQ // P      # 16 query token tiles
EC = E // P        # 4 contraction chunks
HC = 4             # heads per core
MC = 2             # 128-wide col blocks of this core's 256 head dims
SCALE = float(D) ** -0.5
EPS = 1e-5
MBIG = 224.0       # mask offset: exp(s*SCALE + 224*m*SCALE - 28)

_CACHE = {}


def _build(nkt: int):
    """nkt = number of 128-key tiles after compaction (KP = nkt*128)."""
    KP = nkt * P
    # kv token-tile groups of up to 4 (for LN/proj batching)
    kgroups = [list(range(g, min(g + 4, nkt))) for g in range(0, nkt, 4)]
    # exp/score batches of up to 2 k-tiles (sp psum tile is 2 banks)
    kbatches = [list(range(b, min(b + 2, nkt))) for b in range(0, nkt, 2)]

    nc = bass.Bass("TRN2", target_bir_lowering=False, debug=False, num_devices=8)
    f32 = mybir.dt.float32
    bf16 = mybir.dt.bfloat16
    fp8 = mybir.dt.float8e4

    xq = nc.dram_tensor("xq", [TQ, E], f32, kind="ExternalInput").ap()
    xkv = nc.dram_tensor("xkv", [KP, E], f32, kind="ExternalInput").ap()
    wqd = nc.dram_tensor("wq", [E, MC * P], bf16, kind="ExternalInput").ap()
    wkd = nc.dram_tensor("wk", [E, MC * P], bf16, kind="ExternalInput").ap()
    wvd = nc.dram_tensor("wv", [E, MC * P], bf16, kind="ExternalInput").ap()
    wod = nc.dram_tensor("wo", [MC * P, E], bf16, kind="ExternalInput").ap()
    # mask, fp8 {0,1}, [k, q] layout, compacted+padded keys
    m8d = nc.dram_tensor("m8", [KP, TQ], fp8, kind="ExternalInput").ap()
    identd = nc.dram_tensor("ident", [P, P], bf16, kind="ExternalInput").ap()
    # identM: [128, 2, 128] fp8; [:,0,:]=224*I, [:,1,:]=0
    identMd = nc.dram_tensor("identM", [P, 2, P], fp8, kind="ExternalInput").ap()
    outd = nc.dram_tensor("out", [TQ, E], f32, kind="ExternalOutput").ap()

    m8r = m8d.rearrange("(c p) q -> p c q", p=P)

    with _TileContext(nc) as tc:
        with (
            tc.tile_pool(name="persist", bufs=1) as pp,
            tc.tile_pool(name="xs", bufs=5) as xpool,
            tc.tile_pool(name="work", bufs=4) as wk_pool,
            tc.tile_pool(name="scratch", bufs=4) as scratch,
            tc.tile_pool(name="pt", bufs=2) as ptpool,
            tc.tile_pool(name="psA", bufs=2, space="PSUM") as psA,
            tc.tile_pool(name="psS", bufs=2, space="PSUM") as psS,
            tc.tile_pool(name="psC", bufs=2, space="PSUM") as psC,
        ):
            # ---- persistent SBUF tensors ----
            wq_sb = pp.tile([P, EC, MC * P], bf16, tag="wq")
            wk_sb = pp.tile([P, EC, MC * P], bf16, tag="wk")
            wv_sb = pp.tile([P, EC, MC * P], bf16, tag="wv")
            wo_sb = pp.tile([P, MC, E], bf16, tag="wo")
            ident = pp.tile([P, P], bf16, tag="ident")
            identM = pp.tile([P, 2, P], fp8, tag="identM")
            eps_sb = pp.tile([P, 1], f32, tag="eps")
            nb_sb = pp.tile([P, 1], f32, tag="nb")  # -28 exp bias

            m8_sb = [pp.tile([P, len(kb), TQ], fp8, tag=f"m8_{bi}",
                             name=f"m8_{bi}") for bi, kb in enumerate(kbatches)]
            xlnkvT = [pp.tile([P, len(g), EC, P], bf16, tag=f"xkvT{gi}",
                              name=f"xkvT{gi}") for gi, g in enumerate(kgroups)]
            xlnqT = [pp.tile([P, 4, EC, P], bf16, tag=f"xqT{g}",
                             name=f"xqT{g}") for g in range(4)]
            kT_sb = pp.tile([P, MC, KP], bf16, tag="kT")
            qT_g = [pp.tile([P, MC, 512], bf16, tag=f"qT{g}", name=f"qT{g}")
                    for g in range(4)]
            v_gt = pp.tile([P, nkt, HC, D + 1], bf16, tag="v")
            aT_g = [pp.tile([P, MC, 512], bf16, tag=f"aT{g}", name=f"aT{g}")
                    for g in range(4)]
            sums_g = [pp.tile([P, HC * 4], f32, tag=f"sums{g}",
                              name=f"sums{g}") for g in range(4)]
            rsp_g = [pp.tile([P, HC * 4], bf16, tag=f"rsp{g}",
                             name=f"rsp{g}") for g in range(4)]
            rs_flat_g = [pp.tile([1, HC, 512], bf16, tag=f"rsflat{g}",
                                 name=f"rsflat{g}") for g in range(4)]
            ones1 = pp.tile([1, D], bf16, tag="ones1")
            nc.vector.memset(ones1[:], 1.0)

            nc.vector.memset(eps_sb[:], EPS)
            nc.vector.memset(nb_sb[:], -MBIG * SCALE)
            nc.vector.memset(v_gt[:, :, :, D], 1.0)
            nc.sync.dma_start(ident[:], identd)
            nc.sync.dma_start(identM[:], identMd)

            def ln_group(src, dstT, tiles, toff):
                """LN token tiles `tiles` of src into dstT [P, n, EC, P].

                Stats on DVE; apply on GPSIMD (Pool); transpose on PE; the
                psum->SBUF copy on DVE (bf16) or Pool (fp8).
                """
                n = len(tiles)
                mv4 = scratch.tile([P, 4, 2], f32, tag="mv4")
                rsig4 = scratch.tile([P, 4], f32, tag="rsig4")
                sig4 = scratch.tile([P, 4], f32, tag="sig4")
                xts = []
                for i, t in enumerate(tiles):
                    xt = xpool.tile([P, E], f32, tag="x")
                    nc.sync.dma_start(xt[:], src[(toff + t) * P:(toff + t + 1) * P, :])
                    xts.append(xt)
                    stats = scratch.tile([P, 6], f32, tag="bnstats")
                    nc.vector.bn_stats(stats[:], xt[:])
                    nc.vector.bn_aggr(mv4[:, i, :], stats[:])
                # one batched sqrt(var+eps) + reciprocal for the group
                nc.scalar.activation(
                    sig4[:, 0:n], mv4[:, 0:n, 1],
                    mybir.ActivationFunctionType.Sqrt, bias=eps_sb[:])
                nc.vector.reciprocal(rsig4[:, 0:n], sig4[:, 0:n])
                for i, t in enumerate(tiles):
                    xln = wk_pool.tile([P, E], bf16, tag="xln")
                    nc.gpsimd.tensor_scalar(
                        xln[:], xts[i][:], mv4[:, i, 0:1], rsig4[:, i:i + 1],
                        mybir.AluOpType.subtract, mybir.AluOpType.mult)
                    ptr = psA.tile([P, EC, P], bf16, tag="ps")
                    for c in range(EC):
                        nc.tensor.transpose(
                            ptr[:, c, :], xln[:, c * P:(c + 1) * P], ident[:])
                    nc.vector.tensor_copy(dstT[:, i], ptr[:])

            def kproj_group(gi):
                g = kgroups[gi]
                n = len(g)
                for mc in range(MC):
                    ps = psA.tile([P, n * P], f32, tag="ps")
                    for c in range(EC):
                        nc.tensor.matmul(
                            ps[:],
                            lhsT=wk_sb[:, c, mc * P:(mc + 1) * P],
                            rhs=xlnkvT[gi][:, :, c, :],
                            start=(c == 0), stop=(c == EC - 1))
                    nc.vector.tensor_copy(
                        kT_sb[:, mc, g[0] * P:(g[0] + n) * P], ps[:])

            def vproj_group(gi):
                g = kgroups[gi]
                for i, t in enumerate(g):
                    ps = psA.tile([P, MC * P], f32, tag="ps")
                    for c in range(EC):
                        nc.tensor.matmul(
                            ps[:],
                            lhsT=xlnkvT[gi][:, i, c, :],
                            rhs=wv_sb[:, c, :],
                            start=(c == 0), stop=(c == EC - 1))
                    nc.vector.tensor_copy(
                        v_gt[:, t, :, 0:D],
                        ps.rearrange("p (h d) -> p h d", d=D))

            def qproj_group(g):
                for mc in range(MC):
                    ps = psA.tile([P, 512], f32, tag="ps")
                    for c in range(EC):
                        nc.tensor.matmul(
                            ps[:],
                            lhsT=wq_sb[:, c, mc * P:(mc + 1) * P],
                            rhs=xlnqT[g][:, :, c, :],
                            start=(c == 0), stop=(c == EC - 1))
                    nc.vector.tensor_copy(qT_g[g][:, mc, :], ps[:])

            # ---- phase 1: kv side ----
            for gi in range(len(kgroups)):
                ln_group(xkv, xlnkvT[gi], list(range(len(kgroups[gi]))),
                         kgroups[gi][0])
                if gi == 0:
                    nc.sync.dma_start(
                        wk_sb[:], wkd.rearrange("(c p) n -> p c n", p=P))
                    nc.sync.dma_start(
                        wv_sb[:], wvd.rearrange("(c p) n -> p c n", p=P))
                    nc.sync.dma_start(
                        wq_sb[:], wqd.rearrange("(c p) n -> p c n", p=P))
                    nc.sync.dma_start(
                        wo_sb[:], wod.rearrange("(c p) n -> p c n", p=P))
                kproj_group(gi)
                vproj_group(gi)
            for bi, kb in enumerate(kbatches):
                nc.sync.dma_start(m8_sb[bi][:], m8r[:, kb[0]:kb[0] + len(kb), :])

            # ---- phase 2: q group 0, then attention interleaved ----
            ln_group(xq, xlnqT[0], list(range(4)), 0)
            qproj_group(0)

            def attention_qc(qc):
                for h in range(HC):
                    mc = h // 2
                    po = (h % 2) * D
                    pts = {}
                    for bi, kb in enumerate(kbatches):
                        nb2 = len(kb)
                        sp = psS.tile([P, 2, 512], f32, tag="sp")
                        for i, kc in enumerate(kb):
                            nc.tensor.matmul(
                                sp[:, i, :],
                                lhsT=kT_sb[po:po + D, mc, kc * P:(kc + 1) * P],
                                rhs=qT_g[qc][po:po + D, mc, :],
                                start=True, stop=False)
                            nc.tensor.matmul(
                                sp[:, i, :],
                                lhsT=identM[:],
                                rhs=m8_sb[bi][:, i:i + 1, qc * 512:(qc + 1) * 512]
                                    .to_broadcast((P, 2, 512)),
                                start=False, stop=True,
                                perf_mode=mybir.MatmulPerfMode.DoubleRow)
                        pt = ptpool.tile([P, 2, 512], bf16, tag=f"pt{bi}")
                        nc.scalar.activation(
                            pt[:, 0:nb2], sp[:, 0:nb2],
                            mybir.ActivationFunctionType.Exp,
                            bias=nb_sb[:], scale=SCALE)
                        pts[bi] = pt
                    acc = psC.tile([P, 512], f32, tag="acc")
                    for bi, kb in enumerate(kbatches):
                        for i, kc in enumerate(kb):
                            nc.tensor.matmul(
                                acc[:D + 1, :],
                                lhsT=v_gt[:, kc, h, :],
                                rhs=pts[bi][:, i, :],
                                start=(kc == 0), stop=(kc == nkt - 1))
                    stage = wk_pool.tile([P, 512], f32, tag="sumstage")
                    nc.vector.tensor_copy(stage[D:D + 1, :], acc[D:D + 1, :])
                    nc.sync.dma_start(
                        sums_g[qc][:, h * 4:(h + 1) * 4], stage[D:D + 1, :])
                    nc.vector.tensor_copy(
                        aT_g[qc][po:po + D, mc, :], acc[:D])
                    if h % 2 == 1:
                        sl = slice(8 * mc, 8 * mc + 8)
                        nc.vector.tensor_scalar(
                            sums_g[qc][:, sl], sums_g[qc][:, sl], 1e-30, None,
                            mybir.AluOpType.add)
                        rsf = scratch.tile([P, 8], f32, tag="rsf")
                        nc.vector.reciprocal(rsf[:], sums_g[qc][:, sl])
                        nc.vector.tensor_copy(rsp_g[qc][:, sl], rsf[:])
                        for hh in (2 * mc, 2 * mc + 1):
                            nc.sync.dma_start(
                                rs_flat_g[qc][:, hh, :],
                                rsp_g[qc][:, hh * 4:(hh + 1) * 4])
                        rb = psA.tile([P, 512], f32, tag="ps")
                        nc.tensor.matmul(
                            rb[0:D], lhsT=ones1[:],
                            rhs=rs_flat_g[qc][:, 2 * mc, :],
                            start=True, stop=True)
                        nc.tensor.matmul(
                            rb[D:2 * D], lhsT=ones1[:],
                            rhs=rs_flat_g[qc][:, 2 * mc + 1, :],
                            start=True, stop=True)
                        nc.vector.tensor_tensor(
                            aT_g[qc][:, mc, :], aT_g[qc][:, mc, :],
                            rb[:], mybir.AluOpType.mult)

                for ti in range(4):
                    t = qc * 4 + ti
                    ps = psA.tile([P, E], f32, tag="ps")
                    for mc in range(MC):
                        nc.tensor.matmul(
                            ps[:],
                            lhsT=aT_g[qc][:, mc, ti * P:(ti + 1) * P],
                            rhs=wo_sb[:, mc, :],
                            start=(mc == 0), stop=(mc == MC - 1))
                    osb = wk_pool.tile([P, E], f32, tag="osb")
                    nc.vector.tensor_copy(osb[:], ps[:])
                    nc.sync.dma_start(outd[t * P:(t + 1) * P, :], osb[:])

            # ---- phase 1: kv side ----
            for gi in range(len(kgroups)):
                ln_group(xkv, xlnkvT[gi], list(range(len(kgroups[gi]))),
                         kgroups[gi][0])
                if gi == 0:
                    nc.sync.dma_start(
                        wk_sb[:], wkd.rearrange("(c p) n -> p c n", p=P))
                    nc.sync.dma_start(
                        wv_sb[:], wvd.rearrange("(c p) n -> p c n", p=P))
                    nc.sync.dma_start(
                        wq_sb[:], wqd.rearrange("(c p) n -> p c n", p=P))
                    nc.sync.dma_start(
                        wo_sb[:], wod.rearrange("(c p) n -> p c n", p=P))
                kproj_group(gi)
                vproj_group(gi)
            for bi, kb in enumerate(kbatches):
                nc.sync.dma_start(m8_sb[bi][:], m8r[:, kb[0]:kb[0] + len(kb), :])

            # ---- phase 2: q group 0, then attention interleaved ----
            ln_group(xq, xlnqT[0], list(range(4)), 0)
            qproj_group(0)

            for g in range(1, 4):
                ln_group(xq, xlnqT[g], list(range(4)), 4 * g)
                qproj_group(g)
            for qc in range(4):
                attention_qc(qc)

    _split_sync_waits(nc)
    return nc


def _get_nc(nkt: int):
    key = ("nc", nkt)
    if key not in _CACHE:
        _CACHE[key] = _build(nkt)
    return _CACHE[key]


def kernel(query, key_value, kv_mask, sparse_mask,
           ln_q_g, ln_q_b, ln_kv_g, ln_kv_b,
           Wq, bq, Wk, bk, Wv, bv, Wo, bo):
    query = np.asarray(query, np.float32)
    key_value = np.asarray(key_value, np.float32)
    kv_mask = np.asarray(kv_mask)
    sparse_mask = np.asarray(sparse_mask)
    B = query.shape[0]

    # Fold LN gain/bias into the projection weights (exact algebra):
    # (x_ln*g + b) @ W + c  ==  x_ln @ (g[:,None]*W) + (b@W + c)
    Wq_g = np.asarray(ln_q_g, np.float32)[:, None] * np.asarray(Wq, np.float32)
    Wk_g = np.asarray(ln_kv_g, np.float32)[:, None] * np.asarray(Wk, np.float32)
    Wv_g = np.asarray(ln_kv_g, np.float32)[:, None] * np.asarray(Wv, np.float32)
    bq_e = np.asarray(ln_q_b, np.float32) @ np.asarray(Wq, np.float32) + bq
    bk_e = np.asarray(ln_kv_b, np.float32) @ np.asarray(Wk, np.float32) + bk
    bv_e = np.asarray(ln_kv_b, np.float32) @ np.asarray(Wv, np.float32) + bv
    assert not np.any(bq_e) and not np.any(bk_e) and not np.any(bv_e), \
        "nonzero projection biases not supported by this kernel build"

    # ---- key compaction through kv_mask ----
    idxs = [np.where(kv_mask[b])[0] for b in range(B)]
    cnt_max = max(len(ix) for ix in idxs)
    nkt = -(-cnt_max // P)
    KP = nkt * P

    nc = _get_nc(nkt)

    # constants
    identity = np.eye(P, dtype=BF16)
    identM = np.zeros((P, 2, P), dtype=FP8)
    identM[:, 0, :] = (MBIG * np.eye(P)).astype(FP8)

    in_maps = []
    for c in range(8):
        b, hg = c // 2, c % 2
        hs = slice(hg * MC * P, (hg + 1) * MC * P)
        ix = idxs[b]
        ncnt = len(ix)

        xkv_c = np.zeros((KP, E), np.float32)
        xkv_c[:ncnt] = key_value[b][ix]
        m8 = np.zeros((KP, TQ), FP8)
        m8[:ncnt] = sparse_mask[b].T[ix].astype(FP8)

        m = {
            "xq": np.ascontiguousarray(query[b]),
            "xkv": xkv_c,
            "wq": np.ascontiguousarray(Wq_g[:, hs]).astype(BF16),
            "wk": np.ascontiguousarray(Wk_g[:, hs]).astype(BF16),
            "wv": np.ascontiguousarray(Wv_g[:, hs]).astype(BF16),
            "wo": np.ascontiguousarray(np.asarray(Wo, np.float32)[hs, :]).astype(BF16),
            "m8": m8,
            "ident": identity,
            "identM": identM,
        }
        in_maps.append(m)

    res = bass_utils.run_bass_kernel_spmd(
        nc, in_maps, core_ids=list(range(8)),
        trace=bool(os.environ.get("KERNEL_TRACE")))
    globals()["LAST_RESULTS"] = res

    bo_f = np.asarray(bo, np.float32)
    out = np.empty((B, TQ, E), np.float32)
    for b in range(B):
        out[b] = res.results[2 * b]["out"] + res.results[2 * b + 1]["out"] + bo_f
    return out


# revision 15
# speedup vs baseline: 1.3069x; 1.3069x over previous
"""Trainium2 Bass kernel for nn_CrossAttentionLayer (sparse cross attention).

Sharding: 8 cores = 4 batches x 2 head-groups. Core c handles batch c//2 and
heads [4*(c%2), 4*(c%2)+4). Host compacts the key side through kv_mask (the
~50% masked-off keys are dropped and the survivors padded to a multiple of
128), so the device only attends over KP keys.

Device algorithm (per core):
  xlnT     = transpose(layernorm(x))        LN stats on DVE, apply on GPSIMD
                                            (q-side apply emits fp8 directly)
  qT8      = (Wq8.T @ xlnq8T) via fp8 DoubleRow matmuls, J-permuted columns
             so partition 32h+p holds head h, dim 32j+p at free slot j
  kT8      = Wk.T @ xlnkvT (bf16 matmuls, J-permuted), copied to fp8
  v        = xlnkvT.T @ Wv [k, 4h, 65]  (col 64 = 1.0 -> denominator)
  scores   = kT8.T @ qT8 per (head, k-tile, q-block) fp8 DoubleRow matmuls
             + 224*mask accumulated into the same PSUM region (fp8 DR matmul
             against a broadcast mask tile)
  pT       = exp(scores*SCALE - 28) on ACT -> masked-out entries ~e^-28
  acc      = pT.T-chunks @ [v|1]   [q 128, 4h, 65] accumulated over k-tiles
  a        = acc[:, :, 0:64] * (1/acc[:, :, 64])  per-partition normalize
  aT       = transpose(a) via PE; out = aT.T @ Wo  [q, E] partial, f32 to HBM
Host sums the two per-batch partials and adds bo.
"""

import os

import numpy as np
import ml_dtypes

import bass_rust
import concourse.bass as bass
import concourse.mybir as mybir
import concourse.tile as tile
from concourse import bass_utils
from concourse.vector_clock import ScopedClock


class _TileContext(tile.TileContext):
    """TileContext whose kernel-tail drain is split into single-wait drains.

    The walrus build in this environment rejects >1 sync-wait on a Drain
    (CTRL_NO struct): "Too many sync wait commands". The stock
    _drain_and_barrier attaches one wait per outstanding semaphore to a
    single Drain; emit one Drain per wait instead.
    """

    def _drain_and_barrier(self, tick_clock, wait_clock):
        drain_inst = self.nc.sync.drain()
        wait_clock.add_sem_waits(
            drain_inst.ins, ScopedClock({None: tick_clock.global_clock})
        )
        si = drain_inst.ins.sync_info
        if si is not None and si.on_wait and len(si.on_wait) > 1:
            waits = list(si.on_wait)
            drain_inst.ins.sync_info = bass_rust.SyncInfo(
                on_wait=[waits[0]], on_update=si.on_update or [])
            for w in waits[1:]:
                extra = self.nc.sync.drain()
                extra.ins.sync_info = bass_rust.SyncInfo(
                    on_wait=[w], on_update=[])

        self.nc.all_engine_barrier()
        assert self.sems is not None
        popped = self.nc._tile_sem_poison_stack.pop()
        assert popped is self._sem_poison
        self.nc.clear_and_free_semaphores(list(self.sems.allocated().values()))
        self.nc.all_engine_barrier()


def _split_sync_waits(nc):
    """Cap every instruction at one sync wait (walrus build limitation)."""
    for f in nc.m.functions:
        for bb in f.blocks:
            insns = bb.instructions
            out = []
            changed = False
            for ins in insns:
                si = ins.sync_info
                if si is not None and si.on_wait and len(si.on_wait) > 1:
                    waits = list(si.on_wait)
                    for w in waits[:-1]:
                        nop = mybir.InstNoOp(
                            name=nc.get_next_instruction_name(),
                            engine=ins.engine,
                            ins=[], outs=[],
                            sync_info=bass_rust.SyncInfo(
                                on_wait=[w], on_update=[]),
                        )
                        out.append(nop)
                    ins.sync_info = bass_rust.SyncInfo(
                        on_wait=[waits[-1]], on_update=si.on_update or [])
                    changed = True
                out.append(ins)
            if changed:
                bb.instructions = out


BF16 = ml_dtypes.bfloat16
FP8 = ml_dtypes.float8_e4m3

E = 512
H = 8
D = 64
TQ = 2048          # query tokens
P = 128
NQT = TQ // P      # 16 query token tiles
EC = E // P        # 4 contraction chunks
HC = 4             # heads per core
MC = 2             # 128-wide col blocks of this core's 256 head dims
SCALE = float(D) ** -0.5
EPS = 1e-5
MBIG = 224.0       # mask offset: exp(s*SCALE + 224*m*SCALE - 28)

_CACHE = {}


def _build(nkt: int):
    """nkt = number of 128-key tiles after compaction (KP = nkt*128)."""
    KP = nkt * P
    # kv token-tile groups of up to 4 (for LN/proj batching)
    kgroups = [list(range(g, min(g + 4, nkt))) for g in range(0, nkt, 4)]
    # exp/score batches of up to 3 k-tiles (sp psum tile is 3 banks)
    kbatches = [list(range(b, min(b + 3, nkt))) for b in range(0, nkt, 3)]

    nc = bass.Bass("TRN2", target_bir_lowering=False, debug=False, num_devices=8)
    f32 = mybir.dt.float32
    bf16 = mybir.dt.bfloat16
    fp8 = mybir.dt.float8e4

    xq = nc.dram_tensor("xq", [TQ, E], f32, kind="ExternalInput").ap()
    xkv = nc.dram_tensor("xkv", [KP, E], f32, kind="ExternalInput").ap()
    wqd = nc.dram_tensor("wq", [E, MC * P], bf16, kind="ExternalInput").ap()
    wkd = nc.dram_tensor("wk", [E, MC * P], bf16, kind="ExternalInput").ap()
    wvd = nc.dram_tensor("wv", [E, MC * P], bf16, kind="ExternalInput").ap()
    wod = nc.dram_tensor("wo", [MC * P, E], bf16, kind="ExternalInput").ap()
    # mask, fp8 {0,1}, [k, q] layout, compacted+padded keys
    m8d = nc.dram_tensor("m8", [KP, TQ], fp8, kind="ExternalInput").ap()
    identd = nc.dram_tensor("ident", [P, P], bf16, kind="ExternalInput").ap()
    # identM: [128, 2, 128] fp8; [:,0,:]=224*I, [:,1,:]=0
    identMd = nc.dram_tensor("identM", [P, 2, P], fp8, kind="ExternalInput").ap()
    outd = nc.dram_tensor("out", [TQ, E], f32, kind="ExternalOutput").ap()

    m8r = m8d.rearrange("(c p) q -> p c q", p=P)

    with _TileContext(nc) as tc:
        with (
            tc.tile_pool(name="persist", bufs=1) as pp,
            tc.tile_pool(name="xs", bufs=5) as xpool,
            tc.tile_pool(name="work", bufs=4) as wk_pool,
            tc.tile_pool(name="scratch", bufs=4) as scratch,
            tc.tile_pool(name="pt", bufs=2) as ptpool,
            tc.tile_pool(name="psA", bufs=2, space="PSUM") as psA,
            tc.tile_pool(name="psS", bufs=2, space="PSUM") as psS,
        ):
            # ---- persistent SBUF tensors ----
            wq_sb = pp.tile([P, EC, MC * P], bf16, tag="wq")
            wk_sb = pp.tile([P, EC, MC * P], bf16, tag="wk")
            wv_sb = pp.tile([P, EC, MC * P], bf16, tag="wv")
            wo_sb = pp.tile([P, MC, E], bf16, tag="wo")
            ident = pp.tile([P, P], bf16, tag="ident")
            identM = pp.tile([P, 2, P], fp8, tag="identM")
            eps_sb = pp.tile([P, 1], f32, tag="eps")
            nb_sb = pp.tile([P, 1], f32, tag="nb")  # -28 exp bias

            m8_sb = [pp.tile([P, len(kb), TQ], fp8, tag=f"m8_{bi}",
                             name=f"m8_{bi}") for bi, kb in enumerate(kbatches)]
            xlnkvT = [pp.tile([P, len(g), EC, P], bf16, tag=f"xkvT{gi}",
                              name=f"xkvT{gi}") for gi, g in enumerate(kgroups)]
            xlnqT = [pp.tile([P, 4, EC, P], bf16, tag=f"xqT{g}",
                             name=f"xqT{g}") for g in range(4)]
            kT_sb = pp.tile([P, MC, KP], bf16, tag="kT")
            qT_g = [pp.tile([P, MC, 512], bf16, tag=f"qT{g}", name=f"qT{g}")
                    for g in range(4)]
            v_gt = pp.tile([P, nkt, HC, D + 1], bf16, tag="v")
            aT_sb = [pp.tile([P, MC, P], bf16, tag=f"aT{i}", name=f"aT{i}")
                     for i in range(2)]

            nc.vector.memset(eps_sb[:], EPS)
            nc.vector.memset(nb_sb[:], -MBIG * SCALE)
            nc.vector.memset(v_gt[:, :, :, D], 1.0)
            nc.sync.dma_start(ident[:], identd)
            nc.sync.dma_start(identM[:], identMd)

            def ln_group(src, dstT, tiles, toff):
                """LN token tiles `tiles` of src into dstT [P, n, EC, P].

                Stats on DVE; apply on GPSIMD (Pool); transpose on PE; the
                psum->SBUF copy on DVE (bf16) or Pool (fp8).
                """
                n = len(tiles)
                mv4 = scratch.tile([P, 4, 2], f32, tag="mv4")
                rsig4 = scratch.tile([P, 4], f32, tag="rsig4")
                sig4 = scratch.tile([P, 4], f32, tag="sig4")
                xts = []
                for i, t in enumerate(tiles):
                    xt = xpool.tile([P, E], f32, tag="x")
                    nc.sync.dma_start(xt[:], src[(toff + t) * P:(toff + t + 1) * P, :])
                    xts.append(xt)
                    stats = scratch.tile([P, 6], f32, tag="bnstats")
                    nc.vector.bn_stats(stats[:], xt[:])
                    nc.vector.bn_aggr(mv4[:, i, :], stats[:])
                # one batched sqrt(var+eps) + reciprocal for the group
                nc.scalar.activation(
                    sig4[:, 0:n], mv4[:, 0:n, 1],
                    mybir.ActivationFunctionType.Sqrt, bias=eps_sb[:])
                nc.vector.reciprocal(rsig4[:, 0:n], sig4[:, 0:n])
                for i, t in enumerate(tiles):
                    xln = wk_pool.tile([P, E], bf16, tag="xln")
                    nc.gpsimd.tensor_scalar(
                        xln[:], xts[i][:], mv4[:, i, 0:1], rsig4[:, i:i + 1],
                        mybir.AluOpType.subtract, mybir.AluOpType.mult)
                    ptr = psA.tile([P, EC, P], bf16, tag="ps")
                    for c in range(EC):
                        nc.tensor.transpose(
                            ptr[:, c, :], xln[:, c * P:(c + 1) * P], ident[:])
                    nc.vector.tensor_copy(dstT[:, i], ptr[:])

            def kproj_group(gi):
                g = kgroups[gi]
                n = len(g)
                for mc in range(MC):
                    ps = psA.tile([P, n * P], f32, tag="ps")
                    for c in range(EC):
                        nc.tensor.matmul(
                            ps[:],
                            lhsT=wk_sb[:, c, mc * P:(mc + 1) * P],
                            rhs=xlnkvT[gi][:, :, c, :],
                            start=(c == 0), stop=(c == EC - 1))
                    nc.vector.tensor_copy(
                        kT_sb[:, mc, g[0] * P:(g[0] + n) * P], ps[:])

            def vproj_group(gi):
                g = kgroups[gi]
                for i, t in enumerate(g):
                    ps = psA.tile([P, MC * P], f32, tag="ps")
                    for c in range(EC):
                        nc.tensor.matmul(
                            ps[:],
                            lhsT=xlnkvT[gi][:, i, c, :],
                            rhs=wv_sb[:, c, :],
                            start=(c == 0), stop=(c == EC - 1))
                    nc.vector.tensor_copy(
                        v_gt[:, t, :, 0:D],
                        ps.rearrange("p (h d) -> p h d", d=D))

            def qproj_group(g):
                for mc in range(MC):
                    ps = psA.tile([P, 512], f32, tag="ps")
                    for c in range(EC):
                        nc.tensor.matmul(
                            ps[:],
                            lhsT=wq_sb[:, c, mc * P:(mc + 1) * P],
                            rhs=xlnqT[g][:, :, c, :],
                            start=(c == 0), stop=(c == EC - 1))
                    nc.vector.tensor_copy(qT_g[g][:, mc, :], ps[:])

            # ---- phase 1: kv side ----
            for gi in range(len(kgroups)):
                ln_group(xkv, xlnkvT[gi], list(range(len(kgroups[gi]))),
                         kgroups[gi][0])
                if gi == 0:
                    nc.sync.dma_start(
                        wk_sb[:], wkd.rearrange("(c p) n -> p c n", p=P))
                    nc.sync.dma_start(
                        wv_sb[:], wvd.rearrange("(c p) n -> p c n", p=P))
                    nc.sync.dma_start(
                        wq_sb[:], wqd.rearrange("(c p) n -> p c n", p=P))
                    nc.sync.dma_start(
                        wo_sb[:], wod.rearrange("(c p) n -> p c n", p=P))
                kproj_group(gi)
                vproj_group(gi)
            for bi, kb in enumerate(kbatches):
                nc.sync.dma_start(m8_sb[bi][:], m8r[:, kb[0]:kb[0] + len(kb), :])

            # ---- phase 2: q group 0, then attention interleaved ----
            ln_group(xq, xlnqT[0], list(range(4)), 0)
            qproj_group(0)

            def attention_qc(qc):
                pts = {}
                for h in range(HC):
                    mc = h // 2
                    po = (h % 2) * D
                    for bi, kb in enumerate(kbatches):
                        nb = len(kb)
                        sp = psS.tile([P, 3, 512], f32, tag="sp")
                        for i, kc in enumerate(kb):
                            nc.tensor.matmul(
                                sp[:, i, :],
                                lhsT=kT_sb[po:po + D, mc, kc * P:(kc + 1) * P],
                                rhs=qT_g[qc][po:po + D, mc, :],
                                start=True, stop=False)
                            nc.tensor.matmul(
                                sp[:, i, :],
                                lhsT=identM[:],
                                rhs=m8_sb[bi][:, i:i + 1, qc * 512:(qc + 1) * 512]
                                    .to_broadcast((P, 2, 512)),
                                start=False, stop=True,
                                perf_mode=mybir.MatmulPerfMode.DoubleRow)
                        pt = ptpool.tile([P, 3, 512], bf16, tag=f"pt{h}_{bi}")
                        nc.scalar.activation(
                            pt[:, 0:nb], sp[:, 0:nb],
                            mybir.ActivationFunctionType.Exp,
                            bias=nb_sb[:], scale=SCALE)
                        pts[(h, bi)] = pt

                for qt in range(4):
                    acc = psA.tile([P, HC, D + 1], f32, tag="ps")
                    for h in range(HC):
                        for bi, kb in enumerate(kbatches):
                            for i, kc in enumerate(kb):
                                nc.tensor.matmul(
                                    acc[:, h, :],
                                    lhsT=pts[(h, bi)][:, i, qt * P:(qt + 1) * P],
                                    rhs=v_gt[:, kc, h, :],
                                    start=(kc == 0), stop=(kc == nkt - 1))
                    rec = scratch.tile([P, HC], f32, tag="rec")
                    nc.vector.reciprocal(rec[:], acc[:, :, D])
                    a_sb = wk_pool.tile([P, HC, D], bf16, tag="a_sb")
                    nc.vector.tensor_tensor(
                        a_sb[:], acc[:, :, 0:D],
                        rec[:].to_broadcast((P, HC, D)),
                        mybir.AluOpType.mult)
                    trp = psA.tile([P, MC, P], bf16, tag="ps")
                    av = a_sb.rearrange("p h d -> p (h d)")
                    for mc in range(MC):
                        nc.tensor.transpose(
                            trp[:, mc, :], av[:, mc * P:(mc + 1) * P], ident[:])
                    aT = aT_sb[qt % 2]
                    nc.vector.tensor_copy(aT[:], trp[:])
                    op = psA.tile([P, E], f32, tag="ps")
                    for mc in range(MC):
                        nc.tensor.matmul(
                            op[:], lhsT=aT[:, mc, :], rhs=wo_sb[:, mc, :],
                            start=(mc == 0), stop=(mc == MC - 1))
                    osb = wk_pool.tile([P, E], f32, tag="osb")
                    nc.vector.tensor_copy(osb[:], op[:])
                    t = qc * 4 + qt
                    nc.sync.dma_start(outd[t * P:(t + 1) * P, :], osb[:])

            for g in range(1, 4):
                ln_group(xq, xlnqT[g], list(range(4)), 4 * g)
                qproj_group(g)
            for qc in range(4):
                attention_qc(qc)

    _split_sync_waits(nc)
    return nc


def _get_nc(nkt: int):
    key = ("nc", nkt)
    if key not in _CACHE:
        _CACHE[key] = _build(nkt)
    return _CACHE[key]


def kernel(query, key_value, kv_mask, sparse_mask,
           ln_q_g, ln_q_b, ln_kv_g, ln_kv_b,
           Wq, bq, Wk, bk, Wv, bv, Wo, bo):
    query = np.asarray(query, np.float32)
    key_value = np.asarray(key_value, np.float32)
    kv_mask = np.asarray(kv_mask)
    sparse_mask = np.asarray(sparse_mask)
    B = query.shape[0]

    # Fold LN gain/bias into the projection weights (exact algebra):
    # (x_ln*g + b) @ W + c  ==  x_ln @ (g[:,None]*W) + (b@W + c)
    Wq_g = np.asarray(ln_q_g, np.float32)[:, None] * np.asarray(Wq, np.float32)
    Wk_g = np.asarray(ln_kv_g, np.float32)[:, None] * np.asarray(Wk, np.float32)
    Wv_g = np.asarray(ln_kv_g, np.float32)[:, None] * np.asarray(Wv, np.float32)
    bq_e = np.asarray(ln_q_b, np.float32) @ np.asarray(Wq, np.float32) + bq
    bk_e = np.asarray(ln_kv_b, np.float32) @ np.asarray(Wk, np.float32) + bk
    bv_e = np.asarray(ln_kv_b, np.float32) @ np.asarray(Wv, np.float32) + bv
    assert not np.any(bq_e) and not np.any(bk_e) and not np.any(bv_e), \
        "nonzero projection biases not supported by this kernel build"

    # ---- key compaction through kv_mask ----
    idxs = [np.where(kv_mask[b])[0] for b in range(B)]
    cnt_max = max(len(ix) for ix in idxs)
    nkt = -(-cnt_max // P)
    KP = nkt * P

    nc = _get_nc(nkt)

    # constants
    identity = np.eye(P, dtype=BF16)
    identM = np.zeros((P, 2, P), dtype=FP8)
    identM[:, 0, :] = (MBIG * np.eye(P)).astype(FP8)

    in_maps = []
    for c in range(8):
        b, hg = c // 2, c % 2
        hs = slice(hg * MC * P, (hg + 1) * MC * P)
        ix = idxs[b]
        ncnt = len(ix)

        xkv_c = np.zeros((KP, E), np.float32)
        xkv_c[:ncnt] = key_value[b][ix]
        m8 = np.zeros((KP, TQ), FP8)
        m8[:ncnt] = sparse_mask[b].T[ix].astype(FP8)

        m = {
            "xq": np.ascontiguousarray(query[b]),
            "xkv": xkv_c,
            "wq": np.ascontiguousarray(Wq_g[:, hs]).astype(BF16),
            "wk": np.ascontiguousarray(Wk_g[:, hs]).astype(BF16),
            "wv": np.ascontiguousarray(Wv_g[:, hs]).astype(BF16),
            "wo": np.ascontiguousarray(np.asarray(Wo, np.float32)[hs, :]).astype(BF16),
            "m8": m8,
            "ident": identity,
            "identM": identM,
        }
        in_maps.append(m)

    res = bass_utils.run_bass_kernel_spmd(
        nc, in_maps, core_ids=list(range(8)),
        trace=bool(os.environ.get("KERNEL_TRACE")))
    globals()["LAST_RESULTS"] = res

    bo_f = np.asarray(bo, np.float32)
    out = np.empty((B, TQ, E), np.float32)
    for b in range(B):
        out[b] = res.results[2 * b]["out"] + res.results[2 * b + 1]["out"] + bo_f
    return out
